# revision 33
# baseline (speedup 1.0000x reference)
"""Bahdanau (additive) attention kernel for Trainium2, 8 NeuronCores.

Problem: hidden [32,1,256], encoder_outputs [32,4096,256], W_attn [256,512],
b_attn [256], v [256]  ->  context [32,1,256]

  q_b        = hidden_b @ W1^T + b_attn                  (W1 = W_attn[:, :256])
  E_b[s,h]   = tanh(q_b[h] + sum_k enc_b[s,k] W2[h,k])   (W2 = W_attn[:, 256:])
  z_b[s]     = sum_h v[h] E_b[s,h]
  ctx_b[h]   = sum_s exp(z_s) enc_b[s,h] / sum_s exp(z_s)

Sharding: data-parallel over batch, 4 batches per core, params replicated.

Device strategy (per core, per batch):
  - enc is fed from HBM in two bf16 layouts: encT [k-part, s-free] for the
    energy matmul and encN [s-part, h-free] for the context matmul.
  - energy:  psum[E^T] = W2T(stationary) @ encT(moving), f32 accum, through a
    3-deep psum pipe; consts + batch-0's first encT chunk are front-loaded so
    the tanh chain starts early.
  - tanh(+q bias) fused in ScalarE activation, PSUM->SBUF, output bf16.
  - scores:  zT[128,32] psum via Et-block-stationary matmuls with v as the
    1-column moving operand -> scores land sequence-on-partitions.
  - exp on ScalarE with fused per-partition row-sum (accum_out) -> partZ;
    Z_b folded by a 1-col ones matmul into its own psum cell.
  - context: 32 accumulating 1-col-stationary matmuls pT_j @ encN_j into a
    single [1,256] psum row per batch; the batch's raw context and Z_b stream
    out per batch (results DMA as soon as each batch finishes).
  - the final ctx/Z divide runs on the host: the device InstReciprocal
    produced wrong values for [1,1] operands on hardware, and a [B]-length
    f32 divide is free host-side.

The per-batch tail (Z-fold, ctx matmuls, staging) is interleaved into the
next batch's energy slots on the PE so exp->ctx->out never blocks the
energy/tanh pipeline; the last ctx fill segment sits directly before the
final score chunks of each region so the PE engine queue stays fed while
the sequencer waits for the region's last tanh (otherwise each batch
boundary costs a ~1.4us engine stall plus a pstate ramp reset).

bf16 enc quantization keeps end-to-end rel_err ~7.7e-3 (measured vs
reference; gate is 2e-2).
"""

import numpy as np
import ml_dtypes

B, S, H = 32, 4096, 256
NCORES = 8
BL = B // NCORES          # batches per core = 4
KC = H // 128             # feature chunks = 2
SB = S // 128             # sequence blocks = 32
NMM = 512                 # moving cols per energy matmul
FP8 = False                # fp8 encN + residual correction

_CACHE = {}


def _build_nc():
    import concourse.bass as bass
    import concourse.mybir as mybir
    from contextlib import ExitStack

    f32 = mybir.dt.float32
    bf16 = mybir.dt.bfloat16
    i32 = mybir.dt.int32
    fp8 = mybir.dt.float8e4
    AF = mybir.ActivationFunctionType
    ALU = mybir.AluOpType

    nc = bass.Bass()

    QW = H + BL + 2           # qcst packs W1T | hidT | b_attn | ones
    NDT = fp8 if FP8 else bf16
    encT_d = nc.declare_dram_parameter("encT", [BL, KC, 128, S], bf16, isOutput=False)
    # partition-major so each of the 128 DMA rows is SB*H contiguous bytes
    encN_d = nc.declare_dram_parameter("encN", [BL, 128, SB, H], NDT, isOutput=False)
    w2t_d = nc.declare_dram_parameter("w2t", [KC, 128, H], bf16, isOutput=False)
    qcst_d = nc.declare_dram_parameter("qcst", [KC, 128, QW], f32, isOutput=False)
    v_d = nc.declare_dram_parameter("vvec", [128, KC], bf16, isOutput=False)
    if FP8:
        # residual rows bf16(enc) - f32(fp8(enc)), gather-only (never bulk-read)
        encR_d = nc.declare_dram_parameter("encR", [BL * S, H], bf16, isOutput=False)
        icst_d = nc.declare_dram_parameter("icst", [128, 32 + BL], f32, isOutput=False)
    out_d = nc.declare_dram_parameter("out", [1, BL * H + BL], f32, isOutput=True)

    NG = S // 1024            # s-groups per batch = 4 (each 1024 cols)
    NEG = KC * NG             # energy psum groups per batch = 8, i = g*KC+hc

    # ---- software-pipelined PE schedule ----
    # Energy groups lead each region (dense -> tanh chain starts ASAP, e_ps
    # depth-3 pipe stays fed); the previous batch's Z-fold + context matmuls
    # and this batch's score chunks fill the tanh-paced stall slots.
    # cr(p) waits on the gather DMA, so it must come after this batch's last
    # score chunk -- otherwise it head-of-line-blocks exp(b) on the PE stream.
    pe_order = [("q", hc) for hc in range(KC)]
    for b in range(BL):
        p = b - 1
        fills = ([("zb", p), ("cx", p, 0), ("cx", p, 1), ("cx", p, 2),
                  ("cx", p, 3)] if b >= 1 else [])
        f = {2: fills[0:1], 3: fills[1:2], 5: fills[2:3],
             6: fills[3:4], 7: fills[4:5]}
        sc = {3: [("sc", b, 0)], 5: [("sc", b, 1)], 7: [("sc", b, 2)]}
        for i in range(NEG):
            pe_order += [("en", b, i)]
            pe_order += f.get(i, [])
            pe_order += sc.get(i, [])
        pe_order += [("sc", b, NG - 1)]
    pe_order += [("zb", BL - 1), ("cx", BL - 1, 0), ("cx", BL - 1, 1),
                 ("cx", BL - 1, 2), ("cx", BL - 1, 3)]

    act_order = [("qi", hc) for hc in range(KC)]
    for b in range(BL):
        act_order += [("th", b, i) for i in range(NEG)]
        act_order += [("ex", b)]
    # only dveS-incrementing ops get tags
    dve_order = []
    for b in range(BL):
        dve_order += ([("cand", b)] if FP8 else []) + [("res", b)]
    peT = {k: i + 1 for i, k in enumerate(pe_order)}
    actT = {k: i + 1 for i, k in enumerate(act_order)}
    dveT = {k: i + 1 for i, k in enumerate(dve_order)}

    with ExitStack() as ctx:
        E = ctx.enter_context
        # SBUF
        w2t = E(nc.sbuf_tensor("w2t_s", [128, KC, H], bf16))
        qcst = E(nc.sbuf_tensor("qcst_s", [128, KC, QW], f32))
        vt = E(nc.sbuf_tensor("vt_s", [128, KC], bf16))
        q_sb = E(nc.sbuf_tensor("q_sb", [128, KC, BL], f32))
        encT = [E(nc.sbuf_tensor(f"encT{k}", [128, KC, S], bf16)) for k in range(2)]
        encN = [E(nc.sbuf_tensor(f"encN{k}", [128, SB, H], NDT)) for k in range(3)]
        Et = [E(nc.sbuf_tensor(f"Et{k}", [128, KC, S], bf16)) for k in range(2)]
        pT = [E(nc.sbuf_tensor(f"pT{k}", [128, SB], bf16)) for k in range(2)]
        partZ = [E(nc.sbuf_tensor(f"partZ{k}", [128, 1], f32)) for k in range(2)]
        res = E(nc.sbuf_tensor("res", [1, BL * H + BL], f32))
        Zsb = res[0:1, BL * H:BL * H + BL]
        if FP8:
            icst = E(nc.sbuf_tensor("icst_s", [128, 32 + BL], f32))
            mval = [E(nc.sbuf_tensor(f"mval{k}", [128, 1], f32)) for k in range(2)]
            mvalb = [E(nc.sbuf_tensor(f"mvalb{k}", [128, 1], bf16)) for k in range(2)]
            scr0 = E(nc.sbuf_tensor("scr0", [128, SB], f32))
            scr1 = E(nc.sbuf_tensor("scr1", [128, 1], f32))
            offs = [E(nc.sbuf_tensor(f"offs{k}", [128, 1], i32)) for k in range(2)]
            egath = [E(nc.sbuf_tensor(f"egath{k}", [128, H], bf16)) for k in range(2)]
        # PSUM (8 banks): e_ps 3x2 banks (3-deep energy/tanh pipe),
        # zq_ps 1 bank (zT 0:32 | q 32:40 | Zb 40:41), ctx_ps bank (row0 0:256)
        NEPS = 3
        e_ps = [E(nc.psum_tensor(f"e_ps{k}", [128, 1024], f32))
                for k in range(NEPS)]
        zq_ps = E(nc.psum_tensor("zq_ps", [128, 512], f32))
        ctxb_ps = E(nc.psum_tensor("ctxb_ps", [128, 512], f32))
        zT_ps = zq_ps[:, 0:SB]
        Z4_ps = zq_ps[0:1, 40:40 + BL]
        ctx_ps = ctxb_ps[0:1, 0:H]
        # semaphores
        dmaC = E(nc.semaphore("dmaC"))
        dmaT = E(nc.semaphore("dmaT"))
        dmaN = E(nc.semaphore("dmaN"))
        dmaG = E(nc.semaphore("dmaG"))
        dmaO = E(nc.semaphore("dmaO"))
        peS = E(nc.semaphore("peS"))
        actS = E(nc.semaphore("actS"))
        dveS = E(nc.semaphore("dveS"))
        blk = E(nc.Block())

        @blk.sync
        def _(sp):
            # consts first (tiny), then the big enc streams; encT(b) in halves
            # (first half of batch 0 in quarters) so energy starts on partial
            # data.  encN lands before its ctx consumer.  Per-batch outputs
            # ride this queue at the end (head-of-line waits are harmless by
            # then).
            sp.dma_start(out=w2t[:], in_=w2t_d.rearrange("c p h -> p c h")
                         ).then_inc(dmaC, 16)
            sp.dma_start(out=qcst[:], in_=qcst_d.rearrange("c p w -> p c w")
                         ).then_inc(dmaC, 16)
            SHALF = S // 2

            def dma_T(b, c, quarters=False):
                if b >= 2 and c == 0:         # encT slot reused by en(b-2)
                    sp.wait_ge(peS, peT[("en", b - 2, NEG - 1)])
                pieces = 2 if quarters else 1
                for piece in range(pieces):
                    lo = c * SHALF + piece * SHALF // pieces
                    hi = c * SHALF + (piece + 1) * SHALF // pieces
                    sp.dma_start(
                        out=encT[b % 2][:, :, lo:hi],
                        in_=encT_d[b, :, :, lo:hi]
                        .rearrange("c p s -> p c s")
                    ).then_inc(dmaT, 16)

            def dma_N(b):
                if b >= 3:                    # encN slot reused by cx(b-3)
                    sp.wait_ge(peS, peT[("cx", b - 3, 3)])
                sp.dma_start(out=encN[b % 3][:],
                             in_=encN_d[b].rearrange("p j h -> p (j h)")
                             ).then_inc(dmaN, 16)

            dma_T(0, 0, quarters=True)
            sp.dma_start(out=vt[:], in_=v_d[:]).then_inc(dmaC, 16)
            if FP8:
                sp.dma_start(out=icst[:], in_=icst_d[:]).then_inc(dmaC, 16)
            dma_T(0, 1); dma_T(1, 0); dma_T(1, 1)
            dma_N(0)
            dma_T(2, 0); dma_T(2, 1)
            dma_N(1)
            dma_T(3, 0); dma_T(3, 1)
            dma_N(2); dma_N(3)
            for b in range(BL):
                sp.wait_ge(dveS, dveT[("res", b)])
                lo, hi = b * H, (b + 1) * H + (BL if b == BL - 1 else 0)
                sp.dma_start(out=out_d[0:1, lo:hi], in_=res[0:1, lo:hi]
                             ).then_inc(dmaO, 16)
            sp.wait_ge(dmaO, 16 * BL)

        @blk.vector
        def _(dve):
            for b in range(BL):
                if FP8:
                    # candidate chain: per-partition argmax of pT -> residual
                    # row offsets for the gather; subtractive correction means
                    # no masking of pT is needed.
                    if b == 0:
                        dve.wait_ge(dmaC, 64)  # icst
                    dve.wait_ge(actS, actT[("ex", b)])
                    dve.tensor_reduce(mval[b % 2][:], pT[b % 2][:],
                                      mybir.AxisListType.XYZW, ALU.max)
                    dve.tensor_copy(mvalb[b % 2][:], mval[b % 2][:])
                    dve.tensor_scalar(scr0[:], pT[b % 2][:], mval[b % 2][:],
                                      None, ALU.is_equal)
                    dve.tensor_tensor(scr0[:], scr0[:], icst[:, 0:32], ALU.mult)
                    dve.tensor_reduce(scr1[:], scr0[:],
                                      mybir.AxisListType.XYZW, ALU.max)
                    dve.tensor_scalar(scr1[:], scr1[:], 128.0,
                                      icst[:, 32 + b:33 + b],
                                      ALU.mult, ALU.add)
                    dve.tensor_copy(offs[b % 2][:], scr1[:]).then_inc(dveS)
                # stage Z and the raw context; 1/Z is applied on the host
                dve.wait_ge(peS, peT[("zb", b)])
                dve.tensor_copy(Zsb[:, b:b + 1], Z4_ps[:, b:b + 1])
                dve.wait_ge(peS, peT[("cx", b, 3)])
                dve.tensor_copy(res[0:1, b * H:(b + 1) * H], ctx_ps).then_inc(dveS)

        if FP8:
            @blk.gpsimd
            def _(pool):
                for b in range(BL):
                    pool.wait_ge(dveS, dveT[("cand", b)])
                    pool.indirect_dma_start(
                        out=egath[b % 2][:],
                        out_offset=None,
                        in_=encR_d[:],
                        in_offset=bass.IndirectOffsetOnAxis(
                            ap=offs[b % 2][:], axis=0),
                    ).then_inc(dmaG, 16)

        @blk.tensor
        def _(pe):
            pe.wait_ge(dmaC, 32)              # w2t + qcst
            for hc in range(KC):
                for fc in range(KC):
                    mm = pe.matmul(
                        zq_ps[:, 32 + hc * BL:32 + (hc + 1) * BL],
                        qcst[:, fc, hc * 128:(hc + 1) * 128],
                        qcst[:, fc, H:H + BL],
                        start=(fc == 0), stop=(fc == KC - 1))
                mm.then_inc(peS)
            for op in pe_order:
                if op[0] == "q":
                    continue                      # already emitted above
                if op[0] == "en":
                    _, b, i = op
                    g, hc = divmod(i, KC)
                    G = NEG * b + i
                    # dmaT counts pieces: batch 0 = [q1, q2, h2], rest = halves
                    if b == 0:
                        need = {0: 1, 2: 2, 4: 3}.get(i)
                    else:
                        need = ({0: 2 * b + 2, 2 * (NG // 2): 2 * b + 3}
                                .get(i) if (i % KC == 0) else None)
                    if need is not None:
                        pe.wait_ge(dmaT, 16 * need)
                    if G >= NEPS:
                        pb, pi = divmod(G - NEPS, NEG)
                        pe.wait_ge(actS, actT[("th", pb, pi)])
                    for kc in range(KC):
                        for n in range(1024 // NMM):
                            mm = pe.matmul(
                                e_ps[G % NEPS][:, n * NMM:(n + 1) * NMM],
                                w2t[:, kc, hc * 128:(hc + 1) * 128],
                                encT[b % 2][:, kc,
                                            g * 1024 + n * NMM:
                                            g * 1024 + (n + 1) * NMM],
                                start=(kc == 0), stop=(kc == KC - 1))
                    mm.then_inc(peS)
                elif op[0] == "sc":
                    _, b, gq = op
                    if b == 0 and gq == 0:
                        pe.wait_ge(dmaC, 48)  # vt
                        pe.wait_ge(actS, actT[("qi", KC - 1)])
                    pe.wait_ge(actS, actT[("th", b, gq * KC + 1)])
                    for j in range(gq * 8, gq * 8 + 8):
                        for hc in range(KC):
                            mm = pe.matmul(
                                zT_ps[:, j:j + 1],
                                Et[b % 2][:, hc, j * 128:(j + 1) * 128],
                                vt[:, hc:hc + 1],
                                start=(hc == 0), stop=(hc == KC - 1))
                    mm.then_inc(peS)
                elif op[0] == "zb":
                    _, b = op
                    pe.wait_ge(actS, actT[("ex", b)])
                    pe.matmul(Z4_ps[:, b:b + 1], qcst[:, 0, QW - 1:QW],
                              partZ[b % 2][:], start=True, stop=True
                              ).then_inc(peS)
                elif op[0] == "cx":
                    _, b, qr = op
                    if qr == 0:
                        pe.wait_ge(dmaN, 16 * (b + 1))
                        pe.wait_ge(actS, actT[("ex", b)])
                        if b >= 1:
                            pe.wait_ge(dveS, dveT[("res", b - 1)])
                    for j in range(qr * 8, qr * 8 + 8):
                        last = (not FP8) and j == SB - 1
                        mm = pe.matmul(
                            ctx_ps, pT[b % 2][:, j:j + 1],
                            encN[b % 3][:, j, :],
                            start=(j == 0), stop=last)
                    mm.then_inc(peS)

        @blk.scalar
        def _(act):
            act.wait_ge(dmaC, 32)
            for op in act_order:
                if op[0] == "qi":
                    _, hc = op
                    act.wait_ge(peS, peT[("q", hc)])
                    act.activation(q_sb[:, hc, :],
                                   zq_ps[:, 32 + hc * BL:32 + (hc + 1) * BL],
                                   AF.Identity,
                                   bias=qcst[:, hc, H + BL:H + BL + 1]
                                   ).then_inc(actS)
                elif op[0] == "th":
                    _, b, i = op
                    g, hc = divmod(i, KC)
                    if b >= 2 and i == 0:
                        act.wait_ge(peS, peT[("sc", b - 2, NG - 1)])
                    act.wait_ge(peS, peT[("en", b, i)])
                    act.activation(
                        Et[b % 2][:, hc, g * 1024:(g + 1) * 1024],
                        e_ps[(NEG * b + i) % NEPS][:],
                        AF.Tanh, bias=q_sb[:, hc, b:b + 1]).then_inc(actS)
                elif op[0] == "ex":
                    _, b = op
                    act.wait_ge(peS, peT[("sc", b, NG - 1)])
                    if b >= 2:                 # pT/partZ slot reuse
                        act.wait_ge(peS, peT[("cx", b - 2, 3)])
                    act.activation(pT[b % 2][:], zT_ps, AF.Exp,
                                   accum_out=partZ[b % 2][:]).then_inc(actS)

    return nc


def _get_nc():
    if "nc" not in _CACHE:
        _CACHE["nc"] = _build_nc()
    return _CACHE["nc"]


def _make_in_maps(hidden, encoder_outputs, W_attn, b_attn, v):
    bf16 = ml_dtypes.bfloat16
    f8 = ml_dtypes.float8_e4m3          # mybir float8e4's numpy dtype
    hidden = np.asarray(hidden, dtype=np.float32)
    enc = np.asarray(encoder_outputs, dtype=np.float32)
    W_attn = np.asarray(W_attn, dtype=np.float32)
    b_attn = np.asarray(b_attn, dtype=np.float32)
    v = np.asarray(v, dtype=np.float32)

    QW = H + BL + 2
    w2t = np.ascontiguousarray(W_attn[:, H:].T).reshape(KC, 128, H).astype(bf16)
    vv = np.ascontiguousarray(v.reshape(KC, 128).T).astype(bf16)
    if FP8:
        icst = np.empty((128, 32 + BL), dtype=np.float32)
        icst[:, 0:32] = np.arange(32, dtype=np.float32)[None, :]
        icst[:, 32:] = (np.arange(128, dtype=np.float32)[:, None]
                        + S * np.arange(BL, dtype=np.float32)[None, :])

    in_maps = []
    for i in range(NCORES):
        sl = slice(i * BL, (i + 1) * BL)
        enc_sh = enc[sl]                                    # [BL, S, H]
        encT = np.ascontiguousarray(enc_sh.transpose(0, 2, 1)).astype(bf16)
        encT = encT.reshape(BL, KC, 128, S)
        qcst = np.empty((KC, 128, QW), dtype=np.float32)
        qcst[:, :, :H] = W_attn[:, :H].T.reshape(KC, 128, H)
        qcst[:, :, H:H + BL] = hidden[sl, 0, :].T.reshape(KC, 128, BL)
        qcst[:, :, H + BL] = b_attn.reshape(KC, 128)
        qcst[:, :, H + BL + 1] = 1.0
        m = {"encT": encT, "w2t": w2t, "qcst": qcst, "vvec": vv}
        if FP8:
            enc8 = enc_sh.astype(f8)                        # [BL, S, H]
            # partition-major: encN[b, i, j, :] = enc8[b, j*128+i, :]
            m["encN"] = np.ascontiguousarray(
                enc8.reshape(BL, SB, 128, H).transpose(0, 2, 1, 3))
            encb = enc_sh.astype(bf16)
            encR = (encb.astype(np.float32)
                    - enc8.astype(np.float32)).astype(bf16)
            m["encR"] = encR.reshape(BL * S, H)
            m["icst"] = icst
        else:
            m["encN"] = np.ascontiguousarray(
                enc_sh.astype(bf16).reshape(BL, SB, 128, H)
                .transpose(0, 2, 1, 3))
        in_maps.append(m)
    return in_maps


def kernel(hidden, encoder_outputs, W_attn, b_attn, v):
    from concourse.bass_utils import run_bass_kernel_spmd

    nc = _get_nc()
    in_maps = _make_in_maps(hidden, encoder_outputs, W_attn, b_attn, v)
    res = run_bass_kernel_spmd(nc, in_maps, core_ids=list(range(NCORES)))
    outs = []
    for i in range(NCORES):
        flat = np.asarray(res.results[i]["out"], dtype=np.float32)[0]
        o = flat[:BL * H].reshape(BL, H)
        z = flat[BL * H:]
        outs.append(o / z[:, None])
    ctx = np.concatenate(outs, axis=0)                      # [B, H]
    return ctx[:, None, :].astype(np.float32)
